# revision 24
# baseline (speedup 1.0000x reference)
"""Trainium2 Bass kernel for AllegroScalarOutputHead (segment_reduce).

Strategy (8 NeuronCores, SPMD, no collectives):
  - Graphs 4k..4k+3 -> core k (batch is sorted => contiguous node range).
    Edges go to the core that owns their TARGET node.
  - Features shipped transposed in f16 (halves HBM traffic; 1 cyc/row PE).
  - Host precomputes per-edge coefficient c_e = pair_scales[zs*101+zt] *
    atom_scales[zt] and per-node scale/shift lookups (tiny O(E) table reads;
    the TRN2 DGE only supports >=256B row gathers, so elementwise device
    gathers are impractical). All MLP FLOPs and reductions run on device.
  - edge MLP: mm1 = W1e @ x as 2x[128,512] streams per PSUM pair; mm2 =
    W2e^T @ he as [32,512] replicated rows into PSUM quadrants {0,32,64,96}
    x 4 banks (16-supertile sweeps). One contiguous DVE copy moves the sweep
    to SBUF; one SBUF->SBUF DMA re-partitions rows {0,32,64,96} into a
    [128,64] block of the group's u-tile (so vector work uses all lanes).
  - Per-graph reduction: cumulative is_lt masks vs the 4 graph node-id
    boundaries, mask-multiply-reduce into a [128,4] accumulator, one
    final matmul with ones -> [4,1]; host un-diffs and concatenates.
"""

import numpy as np

NCORES = 8
N_NODES = 50000
NUM_GRAPHS = 32
NZ = 101             # atomic-number entries (0..100)
D_NODE = 256
D_EDGE = 128
SUPER = 512          # supertile (matmul moving columns)
UNIT = 4 * SUPER     # pad granularity
SWEEP = 8 * SUPER    # mm2 psum sweep: 8 supertiles = 4096 slots
GROUP = 16 * SWEEP   # u-tile group: 65536 slots
PAD_I = np.int32(1 << 30)

_CACHE = {}


def _sweep_layout(arr_flat, nsw):
    """[nsw*4096] -> [128, nsw*32]: slot n of sweep s -> (n//32, 32*s + n%32)."""
    return np.ascontiguousarray(
        arr_flat.reshape(nsw, 128, 32).transpose(1, 0, 2).reshape(128, nsw * 32)
    )


def _group_layout(arr_flat, ngrp):
    """[ngrp*65536] -> [ngrp*128, 512]: group g rows [128g, 128g+128) hold the
    sweep layout of its 16 sweeps (slot n of sweep s -> (n//32, 32*s + n%32))."""
    return np.ascontiguousarray(
        arr_flat.reshape(ngrp, 16, 128, 32).transpose(0, 2, 1, 3)
        .reshape(ngrp * 128, 512)
    )


def _build(ET, NT):
    """Single merged SPMD program. ET/NT = padded edges/nodes per core."""
    import concourse.bass as bass
    import concourse.tile as tile
    from concourse import bacc, mybir
    from contextlib import ExitStack

    f32 = mybir.dt.float32
    f32r = mybir.dt.float32r
    f16 = mybir.dt.float16
    i32 = mybir.dt.int32
    AF = mybir.ActivationFunctionType
    OP = mybir.AluOpType

    S = ET // SUPER                 # edge supertiles
    NGRP = -(-ET // GROUP)          # edge u-tile groups
    NS = NT // SUPER                # node supertiles
    NSW = -(-NS // 8)               # node sweeps
    NC2 = NSW * 32                  # node u-tile columns
    assert S % 4 == 0 and NS % 4 == 0

    nc = bacc.Bacc("TRN2", debug=False, num_devices=NCORES)

    # ---------------- DRAM parameters --------------------------------------
    eT = nc.declare_dram_parameter("eT", [D_EDGE, ET], f16, isOutput=False)
    CL = nc.declare_dram_parameter("CL", [NGRP * 128, SUPER], f32, isOutput=False)
    itwL = nc.declare_dram_parameter("itwL", [NGRP * 128, SUPER], i32, isOutput=False)
    Brow_d = nc.declare_dram_parameter("Brow", [128, 4], i32, isOutput=False)
    nTa_d = nc.declare_dram_parameter("nTa", [128, NT], f16, isOutput=False)
    nTb_d = nc.declare_dram_parameter("nTb", [128, NT], f16, isOutput=False)
    AL = nc.declare_dram_parameter("AL", [128, NC2], f32, isOutput=False)
    HL = nc.declare_dram_parameter("HL", [128, NC2], f32, isOutput=False)
    idnL = nc.declare_dram_parameter("idnL", [128, NC2], i32, isOutput=False)
    BrowL_d = nc.declare_dram_parameter("BrowL", [128, 4], i32, isOutput=False)
    W1e_d = nc.declare_dram_parameter("W1e", [128, 128], f16, isOutput=False)
    b1e_d = nc.declare_dram_parameter("b1e", [128, 1], f32, isOutput=False)
    W2e_d = nc.declare_dram_parameter("W2e", [128, 32], f16, isOutput=False)
    W1n_d = nc.declare_dram_parameter("W1n", [256, 256], f16, isOutput=False)
    b1n_d = nc.declare_dram_parameter("b1n", [128, 2], f32, isOutput=False)
    W2n_d = nc.declare_dram_parameter("W2n", [128, 64], f16, isOutput=False)
    b2_d = nc.declare_dram_parameter("b2", [128, 2], f32, isOutput=False)  # [b2e,b2n]
    out_d = nc.declare_dram_parameter("out", [4, 1], f32, isOutput=True)

    with tile.TileContext(nc) as tc, ExitStack() as ctx:
        const = ctx.enter_context(tc.tile_pool(name="const", bufs=1))
        xep = ctx.enter_context(tc.tile_pool(name="xep", bufs=3))
        hep = ctx.enter_context(tc.tile_pool(name="hep", bufs=3))
        up = ctx.enter_context(tc.tile_pool(name="up", bufs=2))
        stp = ctx.enter_context(tc.tile_pool(name="stp", bufs=2))
        gscr = ctx.enter_context(tc.tile_pool(name="gscr", bufs=2))
        ps_mm1 = ctx.enter_context(tc.tile_pool(name="ps_mm1", bufs=2, space="PSUM"))
        ps_mm2 = ctx.enter_context(tc.tile_pool(name="ps_mm2", bufs=2, space="PSUM"))

        # ---------------- constants ----------------------------------------
        # first xe block + edge-critical weights go FIRST on the DMA queue so
        # the PE can start within ~3us; everything else trickles in behind.
        XB = 4096  # xe block columns
        xe0 = xep.tile([128, XB], f16, tag="xe")
        nc.sync.dma_start(xe0[:, 0:1024], eT.ap()[:, 0:1024])
        W1e = const.tile([128, 128], f16)
        nc.sync.dma_start(W1e[:], W1e_d.ap())
        b1e = const.tile([128, 1], f32)
        nc.sync.dma_start(b1e[:], b1e_d.ap())
        W2e = const.tile([128, 32], f16)
        nc.sync.dma_start(W2e[:], W2e_d.ap())
        b2 = const.tile([128, 2], f32)
        nc.sync.dma_start(b2[:], b2_d.ap())
        Brow = const.tile([128, 4], i32)
        nc.sync.dma_start(Brow[:], Brow_d.ap())
        BrowL = const.tile([128, 4], i32)
        nc.sync.dma_start(BrowL[:], BrowL_d.ap())
        W1n = []
        for kb in range(2):
            for db in range(2):
                t = const.tile([128, 128], f16, name=f"w1n{kb}{db}")
                nc.sync.dma_start(
                    t[:], W1n_d.ap()[kb * 128:(kb + 1) * 128, db * 128:(db + 1) * 128]
                )
                W1n.append(t)
        b1n = const.tile([128, 2], f32)
        nc.sync.dma_start(b1n[:], b1n_d.ap())
        W2n = const.tile([128, 64], f16)
        nc.sync.dma_start(W2n[:], W2n_d.ap())
        nc.sync.dma_start(xe0[:, 1024:XB], eT.ap()[:, 1024:XB])
        ones_col = const.tile([128, 1], f32)
        nc.vector.memset(ones_col[:], 1.0)

        accE = const.tile([128, 4], f32)
        nc.vector.memset(accE[:], 0.0)
        accN = const.tile([128, 4], f32)
        nc.vector.memset(accN[:], 0.0)

        # node features prefetched in chunks interleaved with late xe blocks
        nTa = const.tile([128, NT], f16)
        nTb = const.tile([128, NT], f16)

        # ---------------- edge stream --------------------------------------
        # mm2 sweep: 16 supertiles -> one [128, 2048] 4-bank psum tile; slot
        # r = 4q+b -> [32q:32q+32, 512b:512b+512] (rows replicated 32x).
        # DVE copies the sweep to SBUF; a strided SBUF->SBUF DMA picks rows
        # {0,32,64,96} (flat: 16x512 slot-major) into u-tile cols
        # [64sw, 64sw+64) as [128, 64] row-major (slot n -> (n//64, n%64)).
        utile = ctile = ititle = pt2 = None
        rows = 0
        NXB = -(-S * SUPER // XB)           # xe blocks
        NTCH = 8                            # nT prefetch chunks
        he_pipe = []                        # (he, s) awaiting mm2

        def mm2_side(s):
            """Emit mm2 + sweep/group bookkeeping for supertile s (>=0)."""
            nonlocal pt2, utile, ctile, ititle, rows
            g, sg = divmod(s, 128)
            sw, r = divmod(s, 8)
            he = he_pipe.pop(0)
            if sg == 0:                     # new group: u/c/itw tiles
                rows = min(128, S - s)      # supertiles in this group
                utile = up.tile([128, SUPER], f32, tag="u")
                ctile = up.tile([128, SUPER], f32, tag="c")
                ititle = up.tile([128, SUPER], i32, tag="it")
                nc.sync.dma_start(ctile[:], CL.ap()[g * 128:g * 128 + 128, :])
                nc.sync.dma_start(ititle[:], itwL.ap()[g * 128:g * 128 + 128, :])
            if r == 0:
                pt2 = ps_mm2.tile([128, 1024], f32, tag="mm2")
                if S - sw * 8 < 8:          # partial sweep: zero unused slots
                    nc.vector.memset(pt2[:], 0.0)
            q, bk = divmod(r, 2)
            nc.tensor.matmul(pt2[32 * q:32 * q + 32, 512 * bk:512 * bk + 512],
                             W2e[:], he, start=True, stop=True,
                             tile_position=(0, 32 * q))
            if r == 7 or s == S - 1:        # sweep done: copy + re-partition
                stag = stp.tile([128, 1024], f32, tag="stag")
                nc.vector.tensor_copy(stag[:], pt2[:])
                uc = (sw % 16) * 32
                nc.sync.dma_start(utile[:, uc:uc + 32], stag[0:128:32, :])
            if sg == 127 or s == S - 1:     # group done: apply c + masks
                LC = (rows + 7) // 8 * 32   # live u-cols: 32 per sweep
                um = gscr.tile([128, SUPER], f32, tag="um")
                nc.vector.scalar_tensor_tensor(
                    um[:, 0:LC], utile[:, 0:LC], b2[:, 0:1], ctile[:, 0:LC],
                    OP.add, OP.mult
                )
                M4 = gscr.tile([128, 4, SUPER], f32, tag="m4")
                nc.vector.tensor_tensor(
                    M4[:, :, 0:LC],
                    ititle[:, 0:LC].unsqueeze(1).broadcast_to([128, 4, LC]),
                    Brow[:].unsqueeze(2).broadcast_to([128, 4, LC]),
                    OP.is_lt,
                )
                zz = gscr.tile([128, 4, SUPER], f32, tag="zz")
                nc.vector.tensor_tensor(
                    zz[:, :, 0:LC],
                    um[:, 0:LC].unsqueeze(1).broadcast_to([128, 4, LC]),
                    M4[:, :, 0:LC], OP.mult,
                )
                racc = gscr.tile([128, 4], f32, tag="racc")
                nc.vector.tensor_reduce(
                    racc[:].unsqueeze(2), zz[:, :, 0:LC],
                    mybir.AxisListType.X, OP.add
                )
                nc.vector.tensor_tensor(accE[:], accE[:], racc[:], OP.add)

        for s in range(0, S, 2):            # mm1 side, one pair ahead of mm2
            g, sg = divmod(s, 128)

            if s % (XB // SUPER) == 0:      # new xe block
                bi = s // (XB // SUPER)
                if bi == 0:
                    xe = xe0
                else:
                    bsz = min(XB, ET - s * SUPER)
                    xe = xep.tile([128, XB], f16, tag="xe")
                    nc.sync.dma_start(
                        xe[:, 0:bsz], eT.ap()[:, s * SUPER:s * SUPER + bsz]
                    )
                if bi >= NXB - NTCH:        # prefetch nT chunk behind late xe
                    ch = bi - (NXB - NTCH)
                    c0, c1 = ch * NT // NTCH, (ch + 1) * NT // NTCH
                    nc.sync.dma_start(nTa[:, c0:c1], nTa_d.ap()[:, c0:c1])
                    nc.sync.dma_start(nTb[:, c0:c1], nTb_d.ap()[:, c0:c1])
            ps1 = ps_mm1.tile([128, 1024], f32, tag="mm1")
            co = (s * SUPER) % XB
            nc.tensor.matmul(ps1[:, 0:512], W1e[:], xe[:, co:co + 512],
                             start=True, stop=True)
            nc.tensor.matmul(ps1[:, 512:1024], W1e[:], xe[:, co + 512:co + 1024],
                             start=True, stop=True)
            he = hep.tile([128, 1024], f16, tag="he")
            nc.scalar.activation(he[:], ps1[:], AF.Silu, bias=b1e[:])
            he_pipe.append(he[:, 0:512])
            he_pipe.append(he[:, 512:1024])
            if s >= 2:                      # mm2s of the PREVIOUS pair
                mm2_side(s - 2)
                mm2_side(s - 1)
        mm2_side(S - 2)
        mm2_side(S - 1)

        # ---------------- node stream --------------------------------------
        untile = const.tile([128, NC2], f32)
        atile = const.tile([128, NC2], f32)
        nc.sync.dma_start(atile[:], AL.ap())
        htile = const.tile([128, NC2], f32)
        nc.sync.dma_start(htile[:], HL.ap())
        intile = const.tile([128, NC2], i32)
        nc.sync.dma_start(intile[:], idnL.ap())

        pt2n = None
        hn = [None, None]
        for j in range(NS):
            sw, r = divmod(j, 8)
            if j % 2 == 0:                  # 2-supertile pair, both halves
                for db in range(2):
                    psn = ps_mm1.tile([128, 1024], f32, tag="mm1")
                    for j2 in range(2):
                        c2 = slice((j + j2) * SUPER, (j + j2 + 1) * SUPER)
                        nc.tensor.matmul(psn[:, j2 * 512:j2 * 512 + 512],
                                         W1n[0 * 2 + db][:], nTa[:, c2],
                                         start=True, stop=False)
                        nc.tensor.matmul(psn[:, j2 * 512:j2 * 512 + 512],
                                         W1n[1 * 2 + db][:], nTb[:, c2],
                                         start=False, stop=True)
                    h = hep.tile([128, 1024], f16, tag="he")
                    nc.scalar.activation(h[:], psn[:], AF.Silu, bias=b1n[:, db:db + 1])
                    hn[db] = h
            if r == 0:
                pt2n = ps_mm2.tile([128, 1024], f32, tag="mm2")
                if NS - sw * 8 < 8:
                    nc.vector.memset(pt2n[:], 0.0)
            q, bk = divmod(r, 2)
            sl = pt2n[32 * q:32 * q + 32, 512 * bk:512 * bk + 512]
            hcols = slice((j % 2) * 512, (j % 2) * 512 + 512)
            nc.tensor.matmul(sl, W2n[:, 0:32], hn[0][:, hcols],
                             start=True, stop=False, tile_position=(0, 32 * q))
            nc.tensor.matmul(sl, W2n[:, 32:64], hn[1][:, hcols],
                             start=False, stop=True, tile_position=(0, 32 * q))
            if r == 7 or j == NS - 1:
                stag = stp.tile([128, 1024], f32, tag="stag")
                nc.vector.tensor_copy(stag[:], pt2n[:])
                uc = sw * 32
                nc.sync.dma_start(untile[:, uc:uc + 32], stag[0:128:32, :])

        # wn = (pe_n + b2n) * ascale[z] + ashift[z]
        wn1 = gscr.tile([128, NC2], f32, tag="um")
        nc.vector.scalar_tensor_tensor(
            wn1[:], untile[:], b2[:, 1:2], atile[:], OP.add, OP.mult
        )
        wn = gscr.tile([128, NC2], f32, tag="wn")
        nc.vector.tensor_tensor(wn[:], wn1[:], htile[:], OP.add)
        M4n = gscr.tile([128, 4, NC2], f32, tag="m4n")
        nc.vector.tensor_tensor(
            M4n[:],
            intile[:].unsqueeze(1).broadcast_to([128, 4, NC2]),
            BrowL[:].unsqueeze(2).broadcast_to([128, 4, NC2]),
            OP.is_lt,
        )
        zzn = gscr.tile([128, 4, NC2], f32, tag="zzn")
        nc.vector.tensor_tensor(
            zzn[:], wn[:].unsqueeze(1).broadcast_to([128, 4, NC2]),
            M4n[:], OP.mult,
        )
        raccn = gscr.tile([128, 4], f32, tag="racc")
        nc.vector.tensor_reduce(
            raccn[:].unsqueeze(2), zzn[:], mybir.AxisListType.X, OP.add
        )
        nc.vector.tensor_tensor(accN[:], accN[:], raccn[:], OP.add)

        # ---------------- finalize -----------------------------------------
        accT = const.tile([128, 4], f32)
        nc.vector.tensor_tensor(accT[:], accE[:], accN[:], OP.add)
        Yps = ps_mm1.tile([4, 1], f32, tag="mm1")
        nc.tensor.matmul(Yps[:], accT[:], ones_col[:], start=True, stop=True)
        ysb = const.tile([4, 1], f32)
        nc.vector.tensor_copy(ysb[:], Yps[:])
        nc.sync.dma_start(out_d.ap(), ysb[:])

    nc.compile()
    return nc


def _shard(inputs):
    f16 = np.float16

    node_feats = np.asarray(inputs["node_feats"], np.float32)
    edge_feats = np.asarray(inputs["edge_feats"], np.float32)
    Z = np.asarray(inputs["atomic_numbers"], np.int64)
    idx_s = np.asarray(inputs["idx_s"], np.int32)
    idx_t = np.asarray(inputs["idx_t"], np.int32)
    batch = np.asarray(inputs["batch"], np.int32)

    bounds = np.searchsorted(batch, np.arange(NUM_GRAPHS + 1)).astype(np.int64)
    g_t = batch[idx_t]
    core_of_edge = (g_t >> 2).astype(np.int32)

    e_counts = np.bincount(core_of_edge, minlength=NCORES)
    ET = int(-(-e_counts.max() // UNIT) * UNIT)
    n_counts = bounds[4 * np.arange(NCORES) + 4] - bounds[4 * np.arange(NCORES)]
    NT = int(-(-n_counts.max() // UNIT) * UNIT)
    NGRP = -(-ET // GROUP)
    NSW = -(-(NT // SUPER) // 8)

    ascale = np.asarray(inputs["atom_scales"], np.float32)[:, 0]
    ashift = np.asarray(inputs["atom_shifts"], np.float32)[:, 0]
    pair = np.asarray(inputs["pair_scales"], np.float32)[:, 0]

    W1e = np.asarray(inputs["W1e"], np.float32).astype(f16)
    b1e = np.asarray(inputs["b1e"], np.float32).reshape(128, 1)
    W2e = np.tile(np.asarray(inputs["W2e"], np.float32).reshape(128, 1),
                  (1, 32)).astype(f16)
    W1n = np.asarray(inputs["W1n"], np.float32).astype(f16)
    b1n = np.ascontiguousarray(np.asarray(inputs["b1n"], np.float32).reshape(2, 128).T)
    W2n_2 = np.asarray(inputs["W2n"], np.float32).reshape(2, 128).T
    W2n = np.concatenate(
        [np.tile(W2n_2[:, 0:1], (1, 32)), np.tile(W2n_2[:, 1:2], (1, 32))], axis=1
    ).astype(f16)
    b2 = np.tile(np.array(
        [[np.asarray(inputs["b2e"], np.float32)[0],
          np.asarray(inputs["b2n"], np.float32)[0]]], np.float32), (128, 1))

    # per-edge coefficient (host table lookup; see module docstring)
    c_all = (pair[Z[idx_s] * NZ + Z[idx_t]] * ascale[Z[idx_t]]).astype(np.float32)

    order = np.argsort(core_of_edge, kind="stable")
    starts = np.searchsorted(core_of_edge, np.arange(NCORES + 1), sorter=order)

    in_maps = []
    for k in range(NCORES):
        n0 = int(bounds[4 * k])
        n1 = int(bounds[4 * k + 4])
        nn = n1 - n0
        sel = order[starts[k]:starts[k + 1]]
        E = sel.size

        eTk = np.zeros((D_EDGE, ET), f16)
        eTk[:, :E] = edge_feats[sel].T
        cpad = np.zeros(NGRP * GROUP, np.float32)
        cpad[:E] = c_all[sel]
        itw = np.full(NGRP * GROUP, PAD_I, np.int32)
        itw[:E] = idx_t[sel]

        nTk = np.zeros((D_NODE, NT), f16)
        nTk[:, :nn] = node_feats[n0:n1].T
        NTW = NSW * SWEEP
        apad = np.zeros(NTW, np.float32)
        apad[:nn] = ascale[Z[n0:n1]]
        hpad = np.zeros(NTW, np.float32)
        hpad[:nn] = ashift[Z[n0:n1]]
        idn = np.full(NTW, PAD_I, np.int32)
        idn[:nn] = np.arange(nn, dtype=np.int32)

        Brow = bounds[[4 * k + 1, 4 * k + 2, 4 * k + 3, 4 * k + 4]].astype(np.int32)
        in_maps.append({
            "eT": eTk,
            "CL": _group_layout(cpad, NGRP),
            "itwL": _group_layout(itw, NGRP),
            "Brow": np.tile(Brow.reshape(1, 4), (128, 1)),
            "nTa": np.ascontiguousarray(nTk[:128]),
            "nTb": np.ascontiguousarray(nTk[128:]),
            "AL": _sweep_layout(apad, NSW),
            "HL": _sweep_layout(hpad, NSW),
            "idnL": _sweep_layout(idn, NSW),
            "BrowL": np.tile((Brow - n0).reshape(1, 4), (128, 1)),
            "W1e": W1e, "b1e": b1e, "W2e": W2e,
            "W1n": W1n, "b1n": b1n, "W2n": W2n, "b2": b2,
        })
    return ET, NT, in_maps


LAST_RES = None


def kernel(**inputs) -> np.ndarray:
    global LAST_RES
    from concourse.bass_utils import run_bass_kernel_spmd

    ET, NT, in_maps = _shard(inputs)
    key = (ET, NT)
    if key not in _CACHE:
        _CACHE[key] = _build(ET, NT)
    nc = _CACHE[key]

    res = run_bass_kernel_spmd(nc, in_maps, core_ids=list(range(NCORES)))
    LAST_RES = res
    Y = np.zeros(NUM_GRAPHS, np.float32)
    for k in range(NCORES):
        yp = np.asarray(res.results[k]["out"]).reshape(4)
        Y[4 * k] = yp[0]
        Y[4 * k + 1] = yp[1] - yp[0]
        Y[4 * k + 2] = yp[2] - yp[1]
        Y[4 * k + 3] = yp[3] - yp[2]
    return Y


# revision 25
# speedup vs baseline: 1.0116x; 1.0116x over previous
"""Trainium2 Bass kernel for AllegroScalarOutputHead (segment_reduce).

Strategy (8 NeuronCores, SPMD, no collectives):
  - Graphs 4k..4k+3 -> core k (batch is sorted => contiguous node range).
    Edges go to the core that owns their TARGET node.
  - Features shipped transposed in f16 (halves HBM traffic; 1 cyc/row PE).
  - Host precomputes per-edge coefficient c_e = pair_scales[zs*101+zt] *
    atom_scales[zt] and per-node scale/shift lookups (tiny O(E) table reads;
    the TRN2 DGE only supports >=256B row gathers, so elementwise device
    gathers are impractical). All MLP FLOPs and reductions run on device.
  - edge MLP: mm1 = W1e @ x as 2x[128,512] streams per PSUM pair; mm2 =
    W2e^T @ he as [32,512] replicated rows into PSUM quadrants {0,32,64,96}
    x 4 banks (16-supertile sweeps). One contiguous DVE copy moves the sweep
    to SBUF; one SBUF->SBUF DMA re-partitions rows {0,32,64,96} into a
    [128,64] block of the group's u-tile (so vector work uses all lanes).
  - Per-graph reduction: cumulative is_lt masks vs the 4 graph node-id
    boundaries, mask-multiply-reduce into a [128,4] accumulator, one
    final matmul with ones -> [4,1]; host un-diffs and concatenates.
"""

import numpy as np

NCORES = 8
N_NODES = 50000
NUM_GRAPHS = 32
NZ = 101             # atomic-number entries (0..100)
D_NODE = 256
D_EDGE = 128
SUPER = 512          # supertile (matmul moving columns)
UNIT = 4 * SUPER     # pad granularity
SWEEP = 8 * SUPER    # mm2 psum sweep: 8 supertiles = 4096 slots
GROUP = 16 * SWEEP   # u-tile group: 65536 slots
PAD_I = np.int32(1 << 30)

_CACHE = {}


def _sweep_layout(arr_flat, nsw):
    """[nsw*4096] -> [128, nsw*32]: slot n of sweep s -> (n//32, 32*s + n%32)."""
    return np.ascontiguousarray(
        arr_flat.reshape(nsw, 128, 32).transpose(1, 0, 2).reshape(128, nsw * 32)
    )


def _group_layout(arr_flat, ngrp):
    """[ngrp*65536] -> [ngrp*128, 512]: group g rows [128g, 128g+128) hold the
    sweep layout of its 16 sweeps (slot n of sweep s -> (n//32, 32*s + n%32))."""
    return np.ascontiguousarray(
        arr_flat.reshape(ngrp, 16, 128, 32).transpose(0, 2, 1, 3)
        .reshape(ngrp * 128, 512)
    )


def _build(ET, NT):
    """Single merged SPMD program. ET/NT = padded edges/nodes per core."""
    import concourse.bass as bass
    import concourse.tile as tile
    from concourse import bacc, mybir
    from contextlib import ExitStack

    f32 = mybir.dt.float32
    f32r = mybir.dt.float32r
    f16 = mybir.dt.float16
    i32 = mybir.dt.int32
    AF = mybir.ActivationFunctionType
    OP = mybir.AluOpType

    S = ET // SUPER                 # edge supertiles
    NGRP = -(-ET // GROUP)          # edge u-tile groups
    NS = NT // SUPER                # node supertiles
    NSW = -(-NS // 8)               # node sweeps
    NC2 = NSW * 32                  # node u-tile columns
    assert S % 4 == 0 and NS % 4 == 0

    nc = bacc.Bacc("TRN2", debug=False, num_devices=NCORES)

    # ---------------- DRAM parameters --------------------------------------
    eT = nc.declare_dram_parameter("eT", [D_EDGE, ET], f16, isOutput=False)
    CL = nc.declare_dram_parameter("CL", [NGRP * 128, SUPER], f32, isOutput=False)
    itwL = nc.declare_dram_parameter("itwL", [NGRP * 128, SUPER], i32, isOutput=False)
    Brow_d = nc.declare_dram_parameter("Brow", [128, 4], i32, isOutput=False)
    nTa_d = nc.declare_dram_parameter("nTa", [128, NT], f16, isOutput=False)
    nTb_d = nc.declare_dram_parameter("nTb", [128, NT], f16, isOutput=False)
    AL = nc.declare_dram_parameter("AL", [128, NC2], f32, isOutput=False)
    HL = nc.declare_dram_parameter("HL", [128, NC2], f32, isOutput=False)
    idnL = nc.declare_dram_parameter("idnL", [128, NC2], i32, isOutput=False)
    BrowL_d = nc.declare_dram_parameter("BrowL", [128, 4], i32, isOutput=False)
    W1e_d = nc.declare_dram_parameter("W1e", [128, 128], f16, isOutput=False)
    b1e_d = nc.declare_dram_parameter("b1e", [128, 1], f32, isOutput=False)
    W2e_d = nc.declare_dram_parameter("W2e", [128, 32], f16, isOutput=False)
    W1n_d = nc.declare_dram_parameter("W1n", [256, 256], f16, isOutput=False)
    b1n_d = nc.declare_dram_parameter("b1n", [128, 2], f32, isOutput=False)
    W2n_d = nc.declare_dram_parameter("W2n", [128, 64], f16, isOutput=False)
    b2_d = nc.declare_dram_parameter("b2", [128, 2], f32, isOutput=False)  # [b2e,b2n]
    out_d = nc.declare_dram_parameter("out", [4, 1], f32, isOutput=True)

    with tile.TileContext(nc) as tc, ExitStack() as ctx:
        const = ctx.enter_context(tc.tile_pool(name="const", bufs=1))
        xep = ctx.enter_context(tc.tile_pool(name="xep", bufs=3))
        hep = ctx.enter_context(tc.tile_pool(name="hep", bufs=3))
        up = ctx.enter_context(tc.tile_pool(name="up", bufs=2))
        stp = ctx.enter_context(tc.tile_pool(name="stp", bufs=2))
        gscr = ctx.enter_context(tc.tile_pool(name="gscr", bufs=2))
        ps_mm1 = ctx.enter_context(tc.tile_pool(name="ps_mm1", bufs=2, space="PSUM"))
        ps_mm2 = ctx.enter_context(tc.tile_pool(name="ps_mm2", bufs=2, space="PSUM"))

        # ---------------- constants ----------------------------------------
        # first xe block + edge-critical weights go FIRST on the DMA queue so
        # the PE can start within ~3us; everything else trickles in behind.
        XB = 4096  # xe block columns
        xe0 = xep.tile([128, XB], f16, tag="xe")
        nc.sync.dma_start(xe0[:, 0:1024], eT.ap()[:, 0:1024])
        W1e = const.tile([128, 128], f16)
        nc.sync.dma_start(W1e[:], W1e_d.ap())
        b1e = const.tile([128, 1], f32)
        nc.sync.dma_start(b1e[:], b1e_d.ap())
        nc.sync.dma_start(xe0[:, 1024:2048], eT.ap()[:, 1024:2048])
        W2e = const.tile([128, 32], f16)
        nc.sync.dma_start(W2e[:], W2e_d.ap())
        b2 = const.tile([128, 2], f32)
        nc.sync.dma_start(b2[:], b2_d.ap())
        nc.sync.dma_start(xe0[:, 2048:XB], eT.ap()[:, 2048:XB])
        Brow = const.tile([128, 4], i32)
        nc.sync.dma_start(Brow[:], Brow_d.ap())
        BrowL = const.tile([128, 4], i32)
        nc.sync.dma_start(BrowL[:], BrowL_d.ap())
        W1n = []
        for kb in range(2):
            for db in range(2):
                t = const.tile([128, 128], f16, name=f"w1n{kb}{db}")
                nc.sync.dma_start(
                    t[:], W1n_d.ap()[kb * 128:(kb + 1) * 128, db * 128:(db + 1) * 128]
                )
                W1n.append(t)
        b1n = const.tile([128, 2], f32)
        nc.sync.dma_start(b1n[:], b1n_d.ap())
        W2n = const.tile([128, 64], f16)
        nc.sync.dma_start(W2n[:], W2n_d.ap())
        ones_col = const.tile([128, 1], f32)
        nc.vector.memset(ones_col[:], 1.0)

        accE = const.tile([128, 4], f32)
        nc.vector.memset(accE[:], 0.0)
        accN = const.tile([128, 4], f32)
        nc.vector.memset(accN[:], 0.0)

        # node features prefetched as per-pair chunk tiles behind late xe
        NPAIR = NS // 2
        nTaC = [const.tile([128, 1024], f16, name=f"nta{j}") for j in range(NPAIR)]
        nTbC = [const.tile([128, 1024], f16, name=f"ntb{j}") for j in range(NPAIR)]

        # ---------------- edge stream --------------------------------------
        # mm2 sweep: 16 supertiles -> one [128, 2048] 4-bank psum tile; slot
        # r = 4q+b -> [32q:32q+32, 512b:512b+512] (rows replicated 32x).
        # DVE copies the sweep to SBUF; a strided SBUF->SBUF DMA picks rows
        # {0,32,64,96} (flat: 16x512 slot-major) into u-tile cols
        # [64sw, 64sw+64) as [128, 64] row-major (slot n -> (n//64, n%64)).
        utile = ctile = ititle = pt2 = None
        rows = 0
        NXB = -(-S * SUPER // XB)           # xe blocks
        NTCH = NS // 2                      # nT prefetch chunks (per pair)
        he_pipe = []                        # (he, s) awaiting mm2

        def mm2_side(s):
            """Emit mm2 + sweep/group bookkeeping for supertile s (>=0)."""
            nonlocal pt2, utile, ctile, ititle, rows
            g, sg = divmod(s, 128)
            sw, r = divmod(s, 8)
            he = he_pipe.pop(0)
            if sg == 0:                     # new group: u/c/itw tiles
                rows = min(128, S - s)      # supertiles in this group
                utile = up.tile([128, SUPER], f32, tag="u")
                ctile = up.tile([128, SUPER], f32, tag="c")
                ititle = up.tile([128, SUPER], i32, tag="it")
            if sg == min(16, rows - 1):     # c/itw load (needed at group end)
                nc.sync.dma_start(ctile[:], CL.ap()[g * 128:g * 128 + 128, :])
                nc.sync.dma_start(ititle[:], itwL.ap()[g * 128:g * 128 + 128, :])
            if r == 0:
                pt2 = ps_mm2.tile([128, 1024], f32, tag="mm2")
                if S - sw * 8 < 8:          # partial sweep: zero unused slots
                    nc.vector.memset(pt2[:], 0.0)
            q, bk = divmod(r, 2)
            nc.tensor.matmul(pt2[32 * q:32 * q + 32, 512 * bk:512 * bk + 512],
                             W2e[:], he, start=True, stop=True,
                             tile_position=(0, 32 * q))
            if r == 7 or s == S - 1:        # sweep done: copy + re-partition
                stag = stp.tile([128, 1024], f32, tag="stag")
                nc.vector.tensor_copy(stag[:], pt2[:])
                uc = (sw % 16) * 32
                nc.sync.dma_start(utile[:, uc:uc + 32], stag[0:128:32, :])
            if sg == 127 or s == S - 1:     # group done: apply c + masks
                LC = (rows + 7) // 8 * 32   # live u-cols: 32 per sweep
                um = gscr.tile([128, SUPER], f32, tag="um")
                nc.vector.scalar_tensor_tensor(
                    um[:, 0:LC], utile[:, 0:LC], b2[:, 0:1], ctile[:, 0:LC],
                    OP.add, OP.mult
                )
                M4 = gscr.tile([128, 4, SUPER], f32, tag="m4")
                nc.vector.tensor_tensor(
                    M4[:, :, 0:LC],
                    ititle[:, 0:LC].unsqueeze(1).broadcast_to([128, 4, LC]),
                    Brow[:].unsqueeze(2).broadcast_to([128, 4, LC]),
                    OP.is_lt,
                )
                zz = gscr.tile([128, 4, SUPER], f32, tag="zz")
                nc.vector.tensor_tensor(
                    zz[:, :, 0:LC],
                    um[:, 0:LC].unsqueeze(1).broadcast_to([128, 4, LC]),
                    M4[:, :, 0:LC], OP.mult,
                )
                racc = gscr.tile([128, 4], f32, tag="racc")
                nc.vector.tensor_reduce(
                    racc[:].unsqueeze(2), zz[:, :, 0:LC],
                    mybir.AxisListType.X, OP.add
                )
                nc.vector.tensor_tensor(accE[:], accE[:], racc[:], OP.add)

        for s in range(0, S, 2):            # mm1 side, one pair ahead of mm2
            g, sg = divmod(s, 128)

            if s % (XB // SUPER) == 0:      # new xe block
                bi = s // (XB // SUPER)
                if bi == 0:
                    xe = xe0
                else:
                    bsz = min(XB, ET - s * SUPER)
                    xe = xep.tile([128, XB], f16, tag="xe")
                    nc.sync.dma_start(
                        xe[:, 0:bsz], eT.ap()[:, s * SUPER:s * SUPER + bsz]
                    )
                if NXB - NTCH - 1 <= bi < NXB - 1:  # prefetch nT pair chunks
                    ch = bi - (NXB - NTCH - 1)
                    c0 = ch * 1024
                    nc.sync.dma_start(nTaC[ch][:], nTa_d.ap()[:, c0:c0 + 1024])
                    nc.sync.dma_start(nTbC[ch][:], nTb_d.ap()[:, c0:c0 + 1024])
            ps1 = ps_mm1.tile([128, 1024], f32, tag="mm1")
            co = (s * SUPER) % XB
            nc.tensor.matmul(ps1[:, 0:512], W1e[:], xe[:, co:co + 512],
                             start=True, stop=True)
            nc.tensor.matmul(ps1[:, 512:1024], W1e[:], xe[:, co + 512:co + 1024],
                             start=True, stop=True)
            he = hep.tile([128, 1024], f16, tag="he")
            nc.scalar.activation(he[:], ps1[:], AF.Silu, bias=b1e[:])
            he_pipe.append(he[:, 0:512])
            he_pipe.append(he[:, 512:1024])
            if s >= 2:                      # mm2s of the PREVIOUS pair
                mm2_side(s - 2)
                mm2_side(s - 1)
        mm2_side(S - 2)
        mm2_side(S - 1)

        # ---------------- node stream --------------------------------------
        untile = const.tile([128, NC2], f32)
        atile = const.tile([128, NC2], f32)
        nc.sync.dma_start(atile[:], AL.ap())
        htile = const.tile([128, NC2], f32)
        nc.sync.dma_start(htile[:], HL.ap())
        intile = const.tile([128, NC2], i32)
        nc.sync.dma_start(intile[:], idnL.ap())

        pt2n = None
        hn = [None, None]
        for j in range(NS):
            sw, r = divmod(j, 8)
            if j % 2 == 0:                  # 2-supertile pair, both halves
                for db in range(2):
                    psn = ps_mm1.tile([128, 1024], f32, tag="mm1")
                    for j2 in range(2):
                        c2 = slice(j2 * SUPER, (j2 + 1) * SUPER)
                        nc.tensor.matmul(psn[:, j2 * 512:j2 * 512 + 512],
                                         W1n[0 * 2 + db][:], nTaC[j // 2][:, c2],
                                         start=True, stop=False)
                        nc.tensor.matmul(psn[:, j2 * 512:j2 * 512 + 512],
                                         W1n[1 * 2 + db][:], nTbC[j // 2][:, c2],
                                         start=False, stop=True)
                    h = hep.tile([128, 1024], f16, tag="he")
                    nc.scalar.activation(h[:], psn[:], AF.Silu, bias=b1n[:, db:db + 1])
                    hn[db] = h
            if r == 0:
                pt2n = ps_mm2.tile([128, 1024], f32, tag="mm2")
                if NS - sw * 8 < 8:
                    nc.vector.memset(pt2n[:], 0.0)
            q, bk = divmod(r, 2)
            sl = pt2n[32 * q:32 * q + 32, 512 * bk:512 * bk + 512]
            hcols = slice((j % 2) * 512, (j % 2) * 512 + 512)
            nc.tensor.matmul(sl, W2n[:, 0:32], hn[0][:, hcols],
                             start=True, stop=False, tile_position=(0, 32 * q))
            nc.tensor.matmul(sl, W2n[:, 32:64], hn[1][:, hcols],
                             start=False, stop=True, tile_position=(0, 32 * q))
            if r == 7 or j == NS - 1:
                stag = stp.tile([128, 1024], f32, tag="stag")
                nc.vector.tensor_copy(stag[:], pt2n[:])
                uc = sw * 32
                nc.sync.dma_start(untile[:, uc:uc + 32], stag[0:128:32, :])

        # wn = (pe_n + b2n) * ascale[z] + ashift[z]
        wn1 = gscr.tile([128, NC2], f32, tag="um")
        nc.vector.scalar_tensor_tensor(
            wn1[:], untile[:], b2[:, 1:2], atile[:], OP.add, OP.mult
        )
        wn = gscr.tile([128, NC2], f32, tag="wn")
        nc.vector.tensor_tensor(wn[:], wn1[:], htile[:], OP.add)
        M4n = gscr.tile([128, 4, NC2], f32, tag="m4n")
        nc.vector.tensor_tensor(
            M4n[:],
            intile[:].unsqueeze(1).broadcast_to([128, 4, NC2]),
            BrowL[:].unsqueeze(2).broadcast_to([128, 4, NC2]),
            OP.is_lt,
        )
        zzn = gscr.tile([128, 4, NC2], f32, tag="zzn")
        nc.vector.tensor_tensor(
            zzn[:], wn[:].unsqueeze(1).broadcast_to([128, 4, NC2]),
            M4n[:], OP.mult,
        )
        raccn = gscr.tile([128, 4], f32, tag="racc")
        nc.vector.tensor_reduce(
            raccn[:].unsqueeze(2), zzn[:], mybir.AxisListType.X, OP.add
        )
        nc.vector.tensor_tensor(accN[:], accN[:], raccn[:], OP.add)

        # ---------------- finalize -----------------------------------------
        accT = const.tile([128, 4], f32)
        nc.vector.tensor_tensor(accT[:], accE[:], accN[:], OP.add)
        Yps = ps_mm1.tile([4, 1], f32, tag="mm1")
        nc.tensor.matmul(Yps[:], accT[:], ones_col[:], start=True, stop=True)
        ysb = const.tile([4, 1], f32)
        nc.vector.tensor_copy(ysb[:], Yps[:])
        nc.sync.dma_start(out_d.ap(), ysb[:])

    nc.compile()
    return nc


def _shard(inputs):
    f16 = np.float16

    node_feats = np.asarray(inputs["node_feats"], np.float32)
    edge_feats = np.asarray(inputs["edge_feats"], np.float32)
    Z = np.asarray(inputs["atomic_numbers"], np.int64)
    idx_s = np.asarray(inputs["idx_s"], np.int32)
    idx_t = np.asarray(inputs["idx_t"], np.int32)
    batch = np.asarray(inputs["batch"], np.int32)

    bounds = np.searchsorted(batch, np.arange(NUM_GRAPHS + 1)).astype(np.int64)
    g_t = batch[idx_t]
    core_of_edge = (g_t >> 2).astype(np.int32)

    e_counts = np.bincount(core_of_edge, minlength=NCORES)
    ET = int(-(-e_counts.max() // UNIT) * UNIT)
    n_counts = bounds[4 * np.arange(NCORES) + 4] - bounds[4 * np.arange(NCORES)]
    NT = int(-(-n_counts.max() // UNIT) * UNIT)
    NGRP = -(-ET // GROUP)
    NSW = -(-(NT // SUPER) // 8)

    ascale = np.asarray(inputs["atom_scales"], np.float32)[:, 0]
    ashift = np.asarray(inputs["atom_shifts"], np.float32)[:, 0]
    pair = np.asarray(inputs["pair_scales"], np.float32)[:, 0]

    W1e = np.asarray(inputs["W1e"], np.float32).astype(f16)
    b1e = np.asarray(inputs["b1e"], np.float32).reshape(128, 1)
    W2e = np.tile(np.asarray(inputs["W2e"], np.float32).reshape(128, 1),
                  (1, 32)).astype(f16)
    W1n = np.asarray(inputs["W1n"], np.float32).astype(f16)
    b1n = np.ascontiguousarray(np.asarray(inputs["b1n"], np.float32).reshape(2, 128).T)
    W2n_2 = np.asarray(inputs["W2n"], np.float32).reshape(2, 128).T
    W2n = np.concatenate(
        [np.tile(W2n_2[:, 0:1], (1, 32)), np.tile(W2n_2[:, 1:2], (1, 32))], axis=1
    ).astype(f16)
    b2 = np.tile(np.array(
        [[np.asarray(inputs["b2e"], np.float32)[0],
          np.asarray(inputs["b2n"], np.float32)[0]]], np.float32), (128, 1))

    # per-edge coefficient (host table lookup; see module docstring)
    c_all = (pair[Z[idx_s] * NZ + Z[idx_t]] * ascale[Z[idx_t]]).astype(np.float32)

    order = np.argsort(core_of_edge, kind="stable")
    starts = np.searchsorted(core_of_edge, np.arange(NCORES + 1), sorter=order)

    in_maps = []
    for k in range(NCORES):
        n0 = int(bounds[4 * k])
        n1 = int(bounds[4 * k + 4])
        nn = n1 - n0
        sel = order[starts[k]:starts[k + 1]]
        E = sel.size

        eTk = np.zeros((D_EDGE, ET), f16)
        eTk[:, :E] = edge_feats[sel].T
        cpad = np.zeros(NGRP * GROUP, np.float32)
        cpad[:E] = c_all[sel]
        itw = np.full(NGRP * GROUP, PAD_I, np.int32)
        itw[:E] = idx_t[sel]

        nTk = np.zeros((D_NODE, NT), f16)
        nTk[:, :nn] = node_feats[n0:n1].T
        NTW = NSW * SWEEP
        apad = np.zeros(NTW, np.float32)
        apad[:nn] = ascale[Z[n0:n1]]
        hpad = np.zeros(NTW, np.float32)
        hpad[:nn] = ashift[Z[n0:n1]]
        idn = np.full(NTW, PAD_I, np.int32)
        idn[:nn] = np.arange(nn, dtype=np.int32)

        Brow = bounds[[4 * k + 1, 4 * k + 2, 4 * k + 3, 4 * k + 4]].astype(np.int32)
        in_maps.append({
            "eT": eTk,
            "CL": _group_layout(cpad, NGRP),
            "itwL": _group_layout(itw, NGRP),
            "Brow": np.tile(Brow.reshape(1, 4), (128, 1)),
            "nTa": np.ascontiguousarray(nTk[:128]),
            "nTb": np.ascontiguousarray(nTk[128:]),
            "AL": _sweep_layout(apad, NSW),
            "HL": _sweep_layout(hpad, NSW),
            "idnL": _sweep_layout(idn, NSW),
            "BrowL": np.tile((Brow - n0).reshape(1, 4), (128, 1)),
            "W1e": W1e, "b1e": b1e, "W2e": W2e,
            "W1n": W1n, "b1n": b1n, "W2n": W2n, "b2": b2,
        })
    return ET, NT, in_maps


LAST_RES = None


def kernel(**inputs) -> np.ndarray:
    global LAST_RES
    from concourse.bass_utils import run_bass_kernel_spmd

    ET, NT, in_maps = _shard(inputs)
    key = (ET, NT)
    if key not in _CACHE:
        _CACHE[key] = _build(ET, NT)
    nc = _CACHE[key]

    res = run_bass_kernel_spmd(nc, in_maps, core_ids=list(range(NCORES)))
    LAST_RES = res
    Y = np.zeros(NUM_GRAPHS, np.float32)
    for k in range(NCORES):
        yp = np.asarray(res.results[k]["out"]).reshape(4)
        Y[4 * k] = yp[0]
        Y[4 * k + 1] = yp[1] - yp[0]
        Y[4 * k + 2] = yp[2] - yp[1]
        Y[4 * k + 3] = yp[3] - yp[2]
    return Y


# revision 26
# speedup vs baseline: 1.0386x; 1.0267x over previous
"""Trainium2 Bass kernel for AllegroScalarOutputHead (segment_reduce).

Strategy (8 NeuronCores, SPMD, no collectives):
  - Graphs 4k..4k+3 -> core k (batch is sorted => contiguous node range).
    Edges go to the core that owns their TARGET node.
  - Features shipped transposed in f16 (halves HBM traffic; 1 cyc/row PE).
  - Host precomputes per-edge coefficient c_e = pair_scales[zs*101+zt] *
    atom_scales[zt] and per-node scale/shift lookups (tiny O(E) table reads;
    the TRN2 DGE only supports >=256B row gathers, so elementwise device
    gathers are impractical). All MLP FLOPs and reductions run on device.
  - edge MLP: mm1 = W1e @ x as 2x[128,512] streams per PSUM pair; mm2 =
    W2e^T @ he as [32,512] replicated rows into PSUM quadrants {0,32,64,96}
    x 4 banks (16-supertile sweeps). One contiguous DVE copy moves the sweep
    to SBUF; one SBUF->SBUF DMA re-partitions rows {0,32,64,96} into a
    [128,64] block of the group's u-tile (so vector work uses all lanes).
  - Per-graph reduction: cumulative is_lt masks vs the 4 graph node-id
    boundaries, mask-multiply-reduce into a [128,4] accumulator, one
    final matmul with ones -> [4,1]; host un-diffs and concatenates.
"""

import numpy as np

NCORES = 8
N_NODES = 50000
NUM_GRAPHS = 32
NZ = 101             # atomic-number entries (0..100)
D_NODE = 256
D_EDGE = 128
SUPER = 512          # supertile (matmul moving columns)
UNIT = 4 * SUPER     # pad granularity
SWEEP = 8 * SUPER    # mm2 psum sweep: 8 supertiles = 4096 slots
GROUP = 16 * SWEEP   # u-tile group: 65536 slots
PAD_I = np.int32(1 << 30)

_CACHE = {}


def _sweep_layout(arr_flat, nsw):
    """[nsw*4096] -> [128, nsw*32]: slot n of sweep s -> (n//32, 32*s + n%32)."""
    return np.ascontiguousarray(
        arr_flat.reshape(nsw, 128, 32).transpose(1, 0, 2).reshape(128, nsw * 32)
    )


def _group_layout(arr_flat, ngrp):
    """[ngrp*65536] -> [ngrp*128, 512]: group g rows [128g, 128g+128) hold the
    sweep layout of its 16 sweeps (slot n of sweep s -> (n//32, 32*s + n%32))."""
    return np.ascontiguousarray(
        arr_flat.reshape(ngrp, 16, 128, 32).transpose(0, 2, 1, 3)
        .reshape(ngrp * 128, 512)
    )


def _build(ET, NT):
    """Single merged SPMD program. ET/NT = padded edges/nodes per core."""
    import concourse.bass as bass
    import concourse.tile as tile
    from concourse import bacc, mybir
    from contextlib import ExitStack

    f32 = mybir.dt.float32
    f32r = mybir.dt.float32r
    f16 = mybir.dt.float16
    i32 = mybir.dt.int32
    AF = mybir.ActivationFunctionType
    OP = mybir.AluOpType

    S = ET // SUPER                 # edge supertiles
    NGRP = -(-ET // GROUP)          # edge u-tile groups
    NS = NT // SUPER                # node supertiles
    NSW = -(-NS // 8)               # node sweeps
    NC2 = NSW * 32                  # node u-tile columns
    assert S % 4 == 0 and NS % 4 == 0

    nc = bacc.Bacc("TRN2", debug=False, num_devices=NCORES)

    # ---------------- DRAM parameters --------------------------------------
    eT = nc.declare_dram_parameter("eT", [D_EDGE, ET], f16, isOutput=False)
    CL = nc.declare_dram_parameter("CL", [NGRP * 128, SUPER], f32, isOutput=False)
    itwL = nc.declare_dram_parameter("itwL", [NGRP * 128, SUPER], i32, isOutput=False)
    Brow_d = nc.declare_dram_parameter("Brow", [128, 4], i32, isOutput=False)
    nTa_d = nc.declare_dram_parameter("nTa", [128, NT], f16, isOutput=False)
    nTb_d = nc.declare_dram_parameter("nTb", [128, NT], f16, isOutput=False)
    AL = nc.declare_dram_parameter("AL", [128, NC2], f32, isOutput=False)
    HL = nc.declare_dram_parameter("HL", [128, NC2], f32, isOutput=False)
    idnL = nc.declare_dram_parameter("idnL", [128, NC2], i32, isOutput=False)
    BrowL_d = nc.declare_dram_parameter("BrowL", [128, 4], i32, isOutput=False)
    W1e_d = nc.declare_dram_parameter("W1e", [128, 128], f16, isOutput=False)
    b1e_d = nc.declare_dram_parameter("b1e", [128, 1], f32, isOutput=False)
    W2e_d = nc.declare_dram_parameter("W2e", [128, 32], f16, isOutput=False)
    W1n_d = nc.declare_dram_parameter("W1n", [256, 256], f16, isOutput=False)
    b1n_d = nc.declare_dram_parameter("b1n", [128, 2], f32, isOutput=False)
    W2n_d = nc.declare_dram_parameter("W2n", [128, 64], f16, isOutput=False)
    b2_d = nc.declare_dram_parameter("b2", [128, 2], f32, isOutput=False)  # [b2e,b2n]
    out_d = nc.declare_dram_parameter("out", [4, 1], f32, isOutput=True)

    with tile.TileContext(nc) as tc, ExitStack() as ctx:
        const = ctx.enter_context(tc.tile_pool(name="const", bufs=1))
        xep = ctx.enter_context(tc.tile_pool(name="xep", bufs=3))
        hep = ctx.enter_context(tc.tile_pool(name="hep", bufs=3))
        up = ctx.enter_context(tc.tile_pool(name="up", bufs=2))
        stp = ctx.enter_context(tc.tile_pool(name="stp", bufs=2))
        gscr = ctx.enter_context(tc.tile_pool(name="gscr", bufs=2))
        ps_mm1 = ctx.enter_context(tc.tile_pool(name="ps_mm1", bufs=2, space="PSUM"))
        ps_mm2 = ctx.enter_context(tc.tile_pool(name="ps_mm2", bufs=2, space="PSUM"))

        # ---------------- constants ----------------------------------------
        # first xe block + edge-critical weights go FIRST on the DMA queue so
        # the PE can start within ~3us; everything else trickles in behind.
        XB = 4096  # xe block columns
        xe0 = xep.tile([128, XB], f16, tag="xe")
        nc.sync.dma_start(xe0[:, 0:1024], eT.ap()[:, 0:1024])
        W1e = const.tile([128, 128], f16)
        nc.sync.dma_start(W1e[:], W1e_d.ap())
        b1e = const.tile([128, 1], f32)
        nc.sync.dma_start(b1e[:], b1e_d.ap())
        nc.sync.dma_start(xe0[:, 1024:2048], eT.ap()[:, 1024:2048])
        W2e = const.tile([128, 32], f16)
        nc.sync.dma_start(W2e[:], W2e_d.ap())
        b2 = const.tile([128, 2], f32)
        nc.sync.dma_start(b2[:], b2_d.ap())
        nc.sync.dma_start(xe0[:, 2048:XB], eT.ap()[:, 2048:XB])
        Brow = const.tile([128, 4], i32)
        nc.sync.dma_start(Brow[:], Brow_d.ap())
        BrowL = const.tile([128, 4], i32)
        nc.sync.dma_start(BrowL[:], BrowL_d.ap())
        W1n = []
        for kb in range(2):
            for db in range(2):
                t = const.tile([128, 128], f16, name=f"w1n{kb}{db}")
                nc.sync.dma_start(
                    t[:], W1n_d.ap()[kb * 128:(kb + 1) * 128, db * 128:(db + 1) * 128]
                )
                W1n.append(t)
        b1n = const.tile([128, 2], f32)
        nc.sync.dma_start(b1n[:], b1n_d.ap())
        W2n = const.tile([128, 64], f16)
        nc.sync.dma_start(W2n[:], W2n_d.ap())
        ones_col = const.tile([128, 1], f32)
        nc.vector.memset(ones_col[:], 1.0)

        accE = const.tile([128, 4], f32)
        nc.vector.memset(accE[:], 0.0)
        accN = const.tile([128, 4], f32)
        nc.vector.memset(accN[:], 0.0)

        # node features prefetched as per-pair chunk tiles behind late xe
        NPAIR = NS // 2
        nTaC = [const.tile([128, 1024], f16, name=f"nta{j}") for j in range(NPAIR)]
        nTbC = [const.tile([128, 1024], f16, name=f"ntb{j}") for j in range(NPAIR)]

        # ---------------- edge stream --------------------------------------
        # mm2 sweep: 16 supertiles -> one [128, 2048] 4-bank psum tile; slot
        # r = 4q+b -> [32q:32q+32, 512b:512b+512] (rows replicated 32x).
        # DVE copies the sweep to SBUF; a strided SBUF->SBUF DMA picks rows
        # {0,32,64,96} (flat: 16x512 slot-major) into u-tile cols
        # [64sw, 64sw+64) as [128, 64] row-major (slot n -> (n//64, n%64)).
        utile = ctile = ititle = pt2 = None
        rows = 0
        NXB = -(-S * SUPER // XB)           # xe blocks
        NTCH = NS // 2                      # nT prefetch chunks (per pair)
        he_pipe = []                        # (he, s) awaiting mm2

        def mm2_side(s):
            """Emit mm2 + sweep/group bookkeeping for supertile s (>=0)."""
            nonlocal pt2, utile, ctile, ititle, rows
            g, sg = divmod(s, 128)
            sw, r = divmod(s, 8)
            he = he_pipe.pop(0)
            if sg == 0:                     # new group: u/c/itw tiles
                rows = min(128, S - s)      # supertiles in this group
                utile = up.tile([128, SUPER], f32, tag="u")
                ctile = up.tile([128, SUPER], f32, tag="c")
                ititle = up.tile([128, SUPER], i32, tag="it")
            if sg == min(16, rows - 1):     # c/itw load (needed at group end)
                nc.sync.dma_start(ctile[:], CL.ap()[g * 128:g * 128 + 128, :])
                nc.sync.dma_start(ititle[:], itwL.ap()[g * 128:g * 128 + 128, :])
            if r == 0:
                pt2 = ps_mm2.tile([128, 1024], f32, tag="mm2")
                if S - sw * 8 < 8:          # partial sweep: zero unused slots
                    nc.vector.memset(pt2[:], 0.0)
            q, bk = divmod(r, 2)
            nc.tensor.matmul(pt2[32 * q:32 * q + 32, 512 * bk:512 * bk + 512],
                             W2e[:], he, start=True, stop=True,
                             tile_position=(0, 32 * q))
            if r == 7 or s == S - 1:        # sweep done: copy + re-partition
                stag = stp.tile([128, 1024], f32, tag="stag")
                nc.vector.tensor_copy(stag[:], pt2[:])
                uc = (sw % 16) * 32
                nc.sync.dma_start(utile[:, uc:uc + 32], stag[0:128:32, :])
            if sg == 127 or s == S - 1:     # group done: apply c + masks
                LC = (rows + 7) // 8 * 32   # live u-cols: 32 per sweep
                um = gscr.tile([128, SUPER], f32, tag="um")
                nc.vector.scalar_tensor_tensor(
                    um[:, 0:LC], utile[:, 0:LC], b2[:, 0:1], ctile[:, 0:LC],
                    OP.add, OP.mult
                )
                M4 = gscr.tile([128, 4, SUPER], f32, tag="m4")
                nc.vector.tensor_tensor(
                    M4[:, :, 0:LC],
                    ititle[:, 0:LC].unsqueeze(1).broadcast_to([128, 4, LC]),
                    Brow[:].unsqueeze(2).broadcast_to([128, 4, LC]),
                    OP.is_lt,
                )
                zz = gscr.tile([128, 4, SUPER], f32, tag="zz")
                nc.vector.tensor_tensor(
                    zz[:, :, 0:LC],
                    um[:, 0:LC].unsqueeze(1).broadcast_to([128, 4, LC]),
                    M4[:, :, 0:LC], OP.mult,
                )
                racc = gscr.tile([128, 4], f32, tag="racc")
                nc.vector.tensor_reduce(
                    racc[:].unsqueeze(2), zz[:, :, 0:LC],
                    mybir.AxisListType.X, OP.add
                )
                nc.vector.tensor_tensor(accE[:], accE[:], racc[:], OP.add)

        def issue_xe(bi):
            """Issue the DMA for xe block bi; returns its tile."""
            c0 = bi * XB
            bsz = min(XB, ET - c0)
            t = xep.tile([128, XB], f16, tag="xe", name=f"xeb{bi}")
            nc.sync.dma_start(t[:, 0:bsz], eT.ap()[:, c0:c0 + bsz])
            return t

        xe_tiles = {0: xe0}
        if NXB > 1:
            xe_tiles[1] = issue_xe(1)       # depth-2 prefetch from the start

        for s in range(0, S, 2):            # mm1 side, one pair ahead of mm2
            g, sg = divmod(s, 128)

            if s % (XB // SUPER) == 0:      # new xe block
                bi = s // (XB // SUPER)
                if bi + 2 < NXB:
                    xe_tiles[bi + 2] = issue_xe(bi + 2)
                xe = xe_tiles.pop(bi)
                if NXB - NTCH - 1 <= bi < NXB - 1:  # prefetch nT pair chunks
                    ch = bi - (NXB - NTCH - 1)
                    c0 = ch * 1024
                    nc.sync.dma_start(nTaC[ch][:], nTa_d.ap()[:, c0:c0 + 1024])
                    nc.sync.dma_start(nTbC[ch][:], nTb_d.ap()[:, c0:c0 + 1024])
            ps1 = ps_mm1.tile([128, 1024], f32, tag="mm1")
            co = (s * SUPER) % XB
            nc.tensor.matmul(ps1[:, 0:512], W1e[:], xe[:, co:co + 512],
                             start=True, stop=True)
            nc.tensor.matmul(ps1[:, 512:1024], W1e[:], xe[:, co + 512:co + 1024],
                             start=True, stop=True)
            he = hep.tile([128, 1024], f16, tag="he")
            nc.scalar.activation(he[:], ps1[:], AF.Silu, bias=b1e[:])
            he_pipe.append(he[:, 0:512])
            he_pipe.append(he[:, 512:1024])
            if s >= 2:                      # mm2s of the PREVIOUS pair
                mm2_side(s - 2)
                mm2_side(s - 1)
        mm2_side(S - 2)
        mm2_side(S - 1)

        # ---------------- node stream --------------------------------------
        untile = const.tile([128, NC2], f32)
        atile = const.tile([128, NC2], f32)
        nc.sync.dma_start(atile[:], AL.ap())
        htile = const.tile([128, NC2], f32)
        nc.sync.dma_start(htile[:], HL.ap())
        intile = const.tile([128, NC2], i32)
        nc.sync.dma_start(intile[:], idnL.ap())

        pt2n = None
        hn = [None, None]
        for j in range(NS):
            sw, r = divmod(j, 8)
            if j % 2 == 0:                  # 2-supertile pair, both halves
                for db in range(2):
                    psn = ps_mm1.tile([128, 1024], f32, tag="mm1")
                    for j2 in range(2):
                        c2 = slice(j2 * SUPER, (j2 + 1) * SUPER)
                        nc.tensor.matmul(psn[:, j2 * 512:j2 * 512 + 512],
                                         W1n[0 * 2 + db][:], nTaC[j // 2][:, c2],
                                         start=True, stop=False)
                        nc.tensor.matmul(psn[:, j2 * 512:j2 * 512 + 512],
                                         W1n[1 * 2 + db][:], nTbC[j // 2][:, c2],
                                         start=False, stop=True)
                    h = hep.tile([128, 1024], f16, tag="he")
                    nc.scalar.activation(h[:], psn[:], AF.Silu, bias=b1n[:, db:db + 1])
                    hn[db] = h
            if r == 0:
                pt2n = ps_mm2.tile([128, 1024], f32, tag="mm2")
                if NS - sw * 8 < 8:
                    nc.vector.memset(pt2n[:], 0.0)
            q, bk = divmod(r, 2)
            sl = pt2n[32 * q:32 * q + 32, 512 * bk:512 * bk + 512]
            hcols = slice((j % 2) * 512, (j % 2) * 512 + 512)
            nc.tensor.matmul(sl, W2n[:, 0:32], hn[0][:, hcols],
                             start=True, stop=False, tile_position=(0, 32 * q))
            nc.tensor.matmul(sl, W2n[:, 32:64], hn[1][:, hcols],
                             start=False, stop=True, tile_position=(0, 32 * q))
            if r == 7 or j == NS - 1:
                stag = stp.tile([128, 1024], f32, tag="stag")
                nc.vector.tensor_copy(stag[:], pt2n[:])
                uc = sw * 32
                nc.sync.dma_start(untile[:, uc:uc + 32], stag[0:128:32, :])

        # wn = (pe_n + b2n) * ascale[z] + ashift[z]
        wn1 = gscr.tile([128, NC2], f32, tag="um")
        nc.vector.scalar_tensor_tensor(
            wn1[:], untile[:], b2[:, 1:2], atile[:], OP.add, OP.mult
        )
        wn = gscr.tile([128, NC2], f32, tag="wn")
        nc.vector.tensor_tensor(wn[:], wn1[:], htile[:], OP.add)
        M4n = gscr.tile([128, 4, NC2], f32, tag="m4n")
        nc.vector.tensor_tensor(
            M4n[:],
            intile[:].unsqueeze(1).broadcast_to([128, 4, NC2]),
            BrowL[:].unsqueeze(2).broadcast_to([128, 4, NC2]),
            OP.is_lt,
        )
        zzn = gscr.tile([128, 4, NC2], f32, tag="zzn")
        nc.vector.tensor_tensor(
            zzn[:], wn[:].unsqueeze(1).broadcast_to([128, 4, NC2]),
            M4n[:], OP.mult,
        )
        raccn = gscr.tile([128, 4], f32, tag="racc")
        nc.vector.tensor_reduce(
            raccn[:].unsqueeze(2), zzn[:], mybir.AxisListType.X, OP.add
        )
        nc.vector.tensor_tensor(accN[:], accN[:], raccn[:], OP.add)

        # ---------------- finalize -----------------------------------------
        accT = const.tile([128, 4], f32)
        nc.vector.tensor_tensor(accT[:], accE[:], accN[:], OP.add)
        Yps = ps_mm1.tile([4, 1], f32, tag="mm1")
        nc.tensor.matmul(Yps[:], accT[:], ones_col[:], start=True, stop=True)
        ysb = const.tile([4, 1], f32)
        nc.vector.tensor_copy(ysb[:], Yps[:])
        nc.sync.dma_start(out_d.ap(), ysb[:])

    nc.compile()
    return nc


def _shard(inputs):
    f16 = np.float16

    node_feats = np.asarray(inputs["node_feats"], np.float32)
    edge_feats = np.asarray(inputs["edge_feats"], np.float32)
    Z = np.asarray(inputs["atomic_numbers"], np.int64)
    idx_s = np.asarray(inputs["idx_s"], np.int32)
    idx_t = np.asarray(inputs["idx_t"], np.int32)
    batch = np.asarray(inputs["batch"], np.int32)

    bounds = np.searchsorted(batch, np.arange(NUM_GRAPHS + 1)).astype(np.int64)
    g_t = batch[idx_t]
    core_of_edge = (g_t >> 2).astype(np.int32)

    e_counts = np.bincount(core_of_edge, minlength=NCORES)
    ET = int(-(-e_counts.max() // UNIT) * UNIT)
    n_counts = bounds[4 * np.arange(NCORES) + 4] - bounds[4 * np.arange(NCORES)]
    NT = int(-(-n_counts.max() // UNIT) * UNIT)
    NGRP = -(-ET // GROUP)
    NSW = -(-(NT // SUPER) // 8)

    ascale = np.asarray(inputs["atom_scales"], np.float32)[:, 0]
    ashift = np.asarray(inputs["atom_shifts"], np.float32)[:, 0]
    pair = np.asarray(inputs["pair_scales"], np.float32)[:, 0]

    W1e = np.asarray(inputs["W1e"], np.float32).astype(f16)
    b1e = np.asarray(inputs["b1e"], np.float32).reshape(128, 1)
    W2e = np.tile(np.asarray(inputs["W2e"], np.float32).reshape(128, 1),
                  (1, 32)).astype(f16)
    W1n = np.asarray(inputs["W1n"], np.float32).astype(f16)
    b1n = np.ascontiguousarray(np.asarray(inputs["b1n"], np.float32).reshape(2, 128).T)
    W2n_2 = np.asarray(inputs["W2n"], np.float32).reshape(2, 128).T
    W2n = np.concatenate(
        [np.tile(W2n_2[:, 0:1], (1, 32)), np.tile(W2n_2[:, 1:2], (1, 32))], axis=1
    ).astype(f16)
    b2 = np.tile(np.array(
        [[np.asarray(inputs["b2e"], np.float32)[0],
          np.asarray(inputs["b2n"], np.float32)[0]]], np.float32), (128, 1))

    # per-edge coefficient (host table lookup; see module docstring)
    c_all = (pair[Z[idx_s] * NZ + Z[idx_t]] * ascale[Z[idx_t]]).astype(np.float32)

    order = np.argsort(core_of_edge, kind="stable")
    starts = np.searchsorted(core_of_edge, np.arange(NCORES + 1), sorter=order)

    in_maps = []
    for k in range(NCORES):
        n0 = int(bounds[4 * k])
        n1 = int(bounds[4 * k + 4])
        nn = n1 - n0
        sel = order[starts[k]:starts[k + 1]]
        E = sel.size

        eTk = np.zeros((D_EDGE, ET), f16)
        eTk[:, :E] = edge_feats[sel].T
        cpad = np.zeros(NGRP * GROUP, np.float32)
        cpad[:E] = c_all[sel]
        itw = np.full(NGRP * GROUP, PAD_I, np.int32)
        itw[:E] = idx_t[sel]

        nTk = np.zeros((D_NODE, NT), f16)
        nTk[:, :nn] = node_feats[n0:n1].T
        NTW = NSW * SWEEP
        apad = np.zeros(NTW, np.float32)
        apad[:nn] = ascale[Z[n0:n1]]
        hpad = np.zeros(NTW, np.float32)
        hpad[:nn] = ashift[Z[n0:n1]]
        idn = np.full(NTW, PAD_I, np.int32)
        idn[:nn] = np.arange(nn, dtype=np.int32)

        Brow = bounds[[4 * k + 1, 4 * k + 2, 4 * k + 3, 4 * k + 4]].astype(np.int32)
        in_maps.append({
            "eT": eTk,
            "CL": _group_layout(cpad, NGRP),
            "itwL": _group_layout(itw, NGRP),
            "Brow": np.tile(Brow.reshape(1, 4), (128, 1)),
            "nTa": np.ascontiguousarray(nTk[:128]),
            "nTb": np.ascontiguousarray(nTk[128:]),
            "AL": _sweep_layout(apad, NSW),
            "HL": _sweep_layout(hpad, NSW),
            "idnL": _sweep_layout(idn, NSW),
            "BrowL": np.tile((Brow - n0).reshape(1, 4), (128, 1)),
            "W1e": W1e, "b1e": b1e, "W2e": W2e,
            "W1n": W1n, "b1n": b1n, "W2n": W2n, "b2": b2,
        })
    return ET, NT, in_maps


LAST_RES = None


def kernel(**inputs) -> np.ndarray:
    global LAST_RES
    from concourse.bass_utils import run_bass_kernel_spmd

    ET, NT, in_maps = _shard(inputs)
    key = (ET, NT)
    if key not in _CACHE:
        _CACHE[key] = _build(ET, NT)
    nc = _CACHE[key]

    res = run_bass_kernel_spmd(nc, in_maps, core_ids=list(range(NCORES)))
    LAST_RES = res
    Y = np.zeros(NUM_GRAPHS, np.float32)
    for k in range(NCORES):
        yp = np.asarray(res.results[k]["out"]).reshape(4)
        Y[4 * k] = yp[0]
        Y[4 * k + 1] = yp[1] - yp[0]
        Y[4 * k + 2] = yp[2] - yp[1]
        Y[4 * k + 3] = yp[3] - yp[2]
    return Y


# revision 31
# speedup vs baseline: 1.1162x; 1.0748x over previous
"""Trainium2 Bass kernel for AllegroScalarOutputHead (segment_reduce).

Strategy (8 NeuronCores, SPMD, no collectives):
  - Graphs 4k..4k+3 -> core k (batch is sorted => contiguous node range).
    Edges go to the core that owns their TARGET node.
  - Features shipped transposed in f16 (halves HBM traffic; 1 cyc/row PE).
  - Host precomputes per-edge coefficient c_e = pair_scales[zs*101+zt] *
    atom_scales[zt] and per-node scale/shift lookups (tiny O(E) table reads;
    the TRN2 DGE only supports >=256B row gathers, so elementwise device
    gathers are impractical). All MLP FLOPs and reductions run on device.
  - edge MLP: mm1 = W1e @ x as 2x[128,512] streams per PSUM pair; mm2 =
    W2e^T @ he as [32,512] replicated rows into PSUM quadrants {0,32,64,96}
    x 4 banks (16-supertile sweeps). One contiguous DVE copy moves the sweep
    to SBUF; one SBUF->SBUF DMA re-partitions rows {0,32,64,96} into a
    [128,64] block of the group's u-tile (so vector work uses all lanes).
  - Per-graph reduction: cumulative is_lt masks vs the 4 graph node-id
    boundaries, mask-multiply-reduce into a [128,4] accumulator, one
    final matmul with ones -> [4,1]; host un-diffs and concatenates.
"""

import numpy as np

NCORES = 8
N_NODES = 50000
NUM_GRAPHS = 32
NZ = 101             # atomic-number entries (0..100)
D_NODE = 256
D_EDGE = 128
SUPER = 512          # supertile (matmul moving columns)
UNIT = 4 * SUPER     # pad granularity
SWEEP = 8 * SUPER    # mm2 psum sweep: 8 supertiles = 4096 slots
GROUP = 16 * SWEEP   # u-tile group: 65536 slots
PAD_I = np.int32(1 << 30)

_CACHE = {}


def _sweep_layout(arr_flat, nsw):
    """[nsw*4096] -> [128, nsw*32]: slot n of sweep s -> (n//32, 32*s + n%32)."""
    return np.ascontiguousarray(
        arr_flat.reshape(nsw, 128, 32).transpose(1, 0, 2).reshape(128, nsw * 32)
    )


def _group_layout(arr_flat, ngrp):
    """[ngrp*65536] -> [ngrp*128, 512]: group g rows [128g, 128g+128) hold the
    sweep layout of its 16 sweeps (slot n of sweep s -> (n//32, 32*s + n%32))."""
    return np.ascontiguousarray(
        arr_flat.reshape(ngrp, 16, 128, 32).transpose(0, 2, 1, 3)
        .reshape(ngrp * 128, 512)
    )


def _build(ET, NT):
    """Single merged SPMD program. ET/NT = padded edges/nodes per core."""
    import concourse.bass as bass
    import concourse.tile as tile
    from concourse import bacc, mybir
    from contextlib import ExitStack

    f32 = mybir.dt.float32
    f32r = mybir.dt.float32r
    f16 = mybir.dt.float16
    i32 = mybir.dt.int32
    AF = mybir.ActivationFunctionType
    OP = mybir.AluOpType

    S = ET // SUPER                 # edge supertiles
    NGRP = -(-ET // GROUP)          # edge u-tile groups
    NS = NT // SUPER                # node supertiles
    NSW = -(-NS // 8)               # node sweeps
    NC2 = NSW * 32                  # node u-tile columns
    assert S % 4 == 0 and NS % 4 == 0

    nc = bacc.Bacc("TRN2", debug=False, num_devices=NCORES)

    # ---------------- DRAM parameters --------------------------------------
    eT = nc.declare_dram_parameter("eT", [D_EDGE, ET], f16, isOutput=False)
    CL = nc.declare_dram_parameter("CL", [NGRP * 128, SUPER], f32, isOutput=False)
    itwL = nc.declare_dram_parameter("itwL", [NGRP * 128, SUPER], i32, isOutput=False)
    Brow_d = nc.declare_dram_parameter("Brow", [128, 4], i32, isOutput=False)
    nTa_d = nc.declare_dram_parameter("nTa", [128, NT], f16, isOutput=False)
    nTb_d = nc.declare_dram_parameter("nTb", [128, NT], f16, isOutput=False)
    AL = nc.declare_dram_parameter("AL", [128, NC2], f32, isOutput=False)
    HL = nc.declare_dram_parameter("HL", [128, NC2], f32, isOutput=False)
    idnL = nc.declare_dram_parameter("idnL", [128, NC2], i32, isOutput=False)
    BrowL_d = nc.declare_dram_parameter("BrowL", [128, 4], i32, isOutput=False)
    W1e_d = nc.declare_dram_parameter("W1e", [128, 128], f16, isOutput=False)
    b1e_d = nc.declare_dram_parameter("b1e", [128, 1], f32, isOutput=False)
    W2e_d = nc.declare_dram_parameter("W2e", [128, 32], f16, isOutput=False)
    W1n_d = nc.declare_dram_parameter("W1n", [256, 256], f16, isOutput=False)
    b1n_d = nc.declare_dram_parameter("b1n", [128, 2], f32, isOutput=False)
    W2n_d = nc.declare_dram_parameter("W2n", [128, 64], f16, isOutput=False)
    b2_d = nc.declare_dram_parameter("b2", [128, 2], f32, isOutput=False)  # [b2e,b2n]
    out_d = nc.declare_dram_parameter("out", [128, 4], f32, isOutput=True)

    with tile.TileContext(nc) as tc, ExitStack() as ctx:
        const = ctx.enter_context(tc.tile_pool(name="const", bufs=1))
        xep = ctx.enter_context(tc.tile_pool(name="xep", bufs=4))
        hep = ctx.enter_context(tc.tile_pool(name="hep", bufs=4))
        up = ctx.enter_context(tc.tile_pool(name="up", bufs=2))
        stp = ctx.enter_context(tc.tile_pool(name="stp", bufs=2))
        gscr = ctx.enter_context(tc.tile_pool(name="gscr", bufs=2))
        ps_mm1 = ctx.enter_context(tc.tile_pool(name="ps_mm1", bufs=2, space="PSUM"))
        ps_mm2 = ctx.enter_context(tc.tile_pool(name="ps_mm2", bufs=2, space="PSUM"))

        # ---------------- constants ----------------------------------------
        # first xe block + edge-critical weights go FIRST on the DMA queue so
        # the PE can start within ~3us; everything else trickles in behind.
        XB = 4096  # xe block columns
        xe0 = xep.tile([128, XB], f16, tag="xe")
        nc.sync.dma_start(xe0[:, 0:1024], eT.ap()[:, 0:1024])
        W1e = const.tile([128, 128], f16)
        nc.sync.dma_start(W1e[:], W1e_d.ap())
        b1e = const.tile([128, 1], f32)
        nc.sync.dma_start(b1e[:], b1e_d.ap())
        nc.sync.dma_start(xe0[:, 1024:2048], eT.ap()[:, 1024:2048])
        W2e = const.tile([128, 32], f16)
        nc.sync.dma_start(W2e[:], W2e_d.ap())
        b2 = const.tile([128, 2], f32)
        nc.sync.dma_start(b2[:], b2_d.ap())
        nc.sync.dma_start(xe0[:, 2048:XB], eT.ap()[:, 2048:XB])
        Brow = const.tile([128, 4], i32)
        nc.sync.dma_start(Brow[:], Brow_d.ap())
        BrowL = const.tile([128, 4], i32)
        nc.sync.dma_start(BrowL[:], BrowL_d.ap())
        xe1 = xep.tile([128, XB], f16, tag="xe", name="xeb1")
        nc.sync.dma_start(xe1[:], eT.ap()[:, XB:2 * XB])
        xe2 = xep.tile([128, XB], f16, tag="xe", name="xeb2")
        nc.sync.dma_start(xe2[:], eT.ap()[:, 2 * XB:3 * XB])
        W1n = []
        for kb in range(2):
            for db in range(2):
                t = const.tile([128, 128], f16, name=f"w1n{kb}{db}")
                nc.sync.dma_start(
                    t[:], W1n_d.ap()[kb * 128:(kb + 1) * 128, db * 128:(db + 1) * 128]
                )
                W1n.append(t)
        b1n = const.tile([128, 2], f32)
        nc.sync.dma_start(b1n[:], b1n_d.ap())
        W2n = const.tile([128, 64], f16)
        nc.sync.dma_start(W2n[:], W2n_d.ap())

        accE = const.tile([128, 4], f32)
        nc.vector.memset(accE[:], 0.0)
        accN = const.tile([128, 4], f32)
        nc.vector.memset(accN[:], 0.0)

        # node features prefetched as per-pair chunk tiles behind late xe
        NPAIR = NS // 2
        nTaC = [const.tile([128, 1024], f16, name=f"nta{j}") for j in range(NPAIR)]
        nTbC = [const.tile([128, 1024], f16, name=f"ntb{j}") for j in range(NPAIR)]

        # ---------------- edge stream --------------------------------------
        # mm2 sweep: 16 supertiles -> one [128, 2048] 4-bank psum tile; slot
        # r = 4q+b -> [32q:32q+32, 512b:512b+512] (rows replicated 32x).
        # DVE copies the sweep to SBUF; a strided SBUF->SBUF DMA picks rows
        # {0,32,64,96} (flat: 16x512 slot-major) into u-tile cols
        # [64sw, 64sw+64) as [128, 64] row-major (slot n -> (n//64, n%64)).
        utile = ctile = ititle = pt2 = None
        rows = 0
        NXB = -(-S * SUPER // XB)           # xe blocks
        NTCH = NS // 2                      # nT prefetch chunks (per pair)
        he_pipe = []                        # (he, s) awaiting mm2

        def mm2_side(s):
            """Emit mm2 + sweep/group bookkeeping for supertile s (>=0)."""
            nonlocal pt2, utile, ctile, ititle, rows
            g, sg = divmod(s, 128)
            sw, r = divmod(s, 8)
            he = he_pipe.pop(0)
            if sg == 0:                     # new group: u/c/itw tiles
                rows = min(128, S - s)      # supertiles in this group
                utile = up.tile([128, SUPER], f32, tag="u")
                ctile = up.tile([128, SUPER], f32, tag="c")
                ititle = up.tile([128, SUPER], i32, tag="it")
            if sg == min(16, rows - 1):     # c/itw load (needed at group end)
                nc.sync.dma_start(ctile[:], CL.ap()[g * 128:g * 128 + 128, :])
                nc.sync.dma_start(ititle[:], itwL.ap()[g * 128:g * 128 + 128, :])
            if r == 0:
                pt2 = ps_mm2.tile([128, 1024], f32, tag="mm2")
                if S - sw * 8 < 8:          # partial sweep: zero unused slots
                    nc.vector.memset(pt2[:], 0.0)
            q, bk = divmod(r, 2)
            nc.tensor.matmul(pt2[32 * q:32 * q + 32, 512 * bk:512 * bk + 512],
                             W2e[:], he, start=True, stop=True,
                             tile_position=(0, 32 * q))
            if r == 7 or s == S - 1:        # sweep done: copy + re-partition
                stag = stp.tile([128, 1024], f32, tag="stag")
                nc.vector.tensor_copy(stag[:], pt2[:])
                uc = (sw % 16) * 32
                nc.sync.dma_start(utile[:, uc:uc + 32], stag[0:128:32, :])
            if sg == 127 or s == S - 1:     # group done: apply c + masks
                LC = (rows + 7) // 8 * 32   # live u-cols: 32 per sweep
                um = gscr.tile([128, SUPER], f32, tag="um")
                nc.vector.scalar_tensor_tensor(
                    um[:, 0:LC], utile[:, 0:LC], b2[:, 0:1], ctile[:, 0:LC],
                    OP.add, OP.mult
                )
                M4 = gscr.tile([128, 4, SUPER], f32, tag="m4")
                nc.vector.tensor_tensor(
                    M4[:, :, 0:LC],
                    ititle[:, 0:LC].unsqueeze(1).broadcast_to([128, 4, LC]),
                    Brow[:].unsqueeze(2).broadcast_to([128, 4, LC]),
                    OP.is_lt,
                )
                zz = gscr.tile([128, 4, SUPER], f32, tag="zz")
                nc.vector.tensor_tensor(
                    zz[:, :, 0:LC],
                    um[:, 0:LC].unsqueeze(1).broadcast_to([128, 4, LC]),
                    M4[:, :, 0:LC], OP.mult,
                )
                racc = gscr.tile([128, 4], f32, tag="racc")
                nc.vector.tensor_reduce(
                    racc[:].unsqueeze(2), zz[:, :, 0:LC],
                    mybir.AxisListType.X, OP.add
                )
                nc.vector.tensor_tensor(accE[:], accE[:], racc[:], OP.add)

        def issue_xe(bi):
            """Issue the DMA for xe block bi; returns its tile."""
            c0 = bi * XB
            bsz = min(XB, ET - c0)
            t = xep.tile([128, XB], f16, tag="xe", name=f"xeb{bi}")
            nc.sync.dma_start(t[:, 0:bsz], eT.ap()[:, c0:c0 + bsz])
            return t

        xe_tiles = {0: xe0, 1: xe1, 2: xe2}

        for s in range(0, S, 2):            # mm1 side, one pair ahead of mm2
            g, sg = divmod(s, 128)

            if s % (XB // SUPER) == 0:      # new xe block
                bi = s // (XB // SUPER)
                if bi + 3 < NXB:
                    xe_tiles[bi + 3] = issue_xe(bi + 3)
                xe = xe_tiles.pop(bi)
                if NXB - NTCH - 1 <= bi < NXB - 1:  # prefetch nT pair chunks
                    ch = bi - (NXB - NTCH - 1)
                    c0 = ch * 1024
                    nc.sync.dma_start(nTaC[ch][:], nTa_d.ap()[:, c0:c0 + 1024])
                    nc.sync.dma_start(nTbC[ch][:], nTb_d.ap()[:, c0:c0 + 1024])
            ps1 = ps_mm1.tile([128, 1024], f32, tag="mm1")
            co = (s * SUPER) % XB
            nc.tensor.matmul(ps1[:, 0:512], W1e[:], xe[:, co:co + 512],
                             start=True, stop=True)
            nc.tensor.matmul(ps1[:, 512:1024], W1e[:], xe[:, co + 512:co + 1024],
                             start=True, stop=True)
            he = hep.tile([128, 1024], f16, tag="he")
            nc.scalar.activation(he[:], ps1[:], AF.Silu, bias=b1e[:])
            he_pipe.append(he[:, 0:512])
            he_pipe.append(he[:, 512:1024])
            if s >= 4:                      # mm2s, two pairs behind mm1
                mm2_side(s - 4)
                mm2_side(s - 3)
        mm2_side(S - 4)
        mm2_side(S - 3)
        mm2_side(S - 2)
        mm2_side(S - 1)

        # ---------------- node stream --------------------------------------
        untile = const.tile([128, NC2], f32)
        atile = const.tile([128, NC2], f32)
        nc.sync.dma_start(atile[:], AL.ap())
        htile = const.tile([128, NC2], f32)
        nc.sync.dma_start(htile[:], HL.ap())
        intile = const.tile([128, NC2], i32)
        nc.sync.dma_start(intile[:], idnL.ap())

        pt2n = None
        hn = [None, None]
        for j in range(NS):
            sw, r = divmod(j, 8)
            if j % 2 == 0:                  # 2-supertile pair, both halves
                for db in range(2):
                    psn = ps_mm1.tile([128, 1024], f32, tag="mm1")
                    for j2 in range(2):
                        c2 = slice(j2 * SUPER, (j2 + 1) * SUPER)
                        nc.tensor.matmul(psn[:, j2 * 512:j2 * 512 + 512],
                                         W1n[0 * 2 + db][:], nTaC[j // 2][:, c2],
                                         start=True, stop=False)
                        nc.tensor.matmul(psn[:, j2 * 512:j2 * 512 + 512],
                                         W1n[1 * 2 + db][:], nTbC[j // 2][:, c2],
                                         start=False, stop=True)
                    h = hep.tile([128, 1024], f16, tag="he")
                    nc.scalar.activation(h[:], psn[:], AF.Silu, bias=b1n[:, db:db + 1])
                    hn[db] = h
            if r == 0:
                pt2n = ps_mm2.tile([128, 1024], f32, tag="mm2")
                if NS - sw * 8 < 8:
                    nc.vector.memset(pt2n[:], 0.0)
            q, bk = divmod(r, 2)
            sl = pt2n[32 * q:32 * q + 32, 512 * bk:512 * bk + 512]
            hcols = slice((j % 2) * 512, (j % 2) * 512 + 512)
            nc.tensor.matmul(sl, W2n[:, 0:32], hn[0][:, hcols],
                             start=True, stop=False, tile_position=(0, 32 * q))
            nc.tensor.matmul(sl, W2n[:, 32:64], hn[1][:, hcols],
                             start=False, stop=True, tile_position=(0, 32 * q))
            if r == 7 or j == NS - 1:
                stag = stp.tile([128, 1024], f32, tag="stag")
                nc.vector.tensor_copy(stag[:], pt2n[:])
                uc = sw * 32
                nc.sync.dma_start(untile[:, uc:uc + 32], stag[0:128:32, :])

        # wn = (pe_n + b2n) * ascale[z] + ashift[z]
        wn1 = gscr.tile([128, NC2], f32, tag="um")
        nc.vector.scalar_tensor_tensor(
            wn1[:], untile[:], b2[:, 1:2], atile[:], OP.add, OP.mult
        )
        wn = gscr.tile([128, NC2], f32, tag="wn")
        nc.vector.tensor_tensor(wn[:], wn1[:], htile[:], OP.add)
        M4n = gscr.tile([128, 4, NC2], f32, tag="m4n")
        nc.vector.tensor_tensor(
            M4n[:],
            intile[:].unsqueeze(1).broadcast_to([128, 4, NC2]),
            BrowL[:].unsqueeze(2).broadcast_to([128, 4, NC2]),
            OP.is_lt,
        )
        zzn = gscr.tile([128, 4, NC2], f32, tag="zzn")
        nc.vector.tensor_tensor(
            zzn[:], wn[:].unsqueeze(1).broadcast_to([128, 4, NC2]),
            M4n[:], OP.mult,
        )
        raccn = gscr.tile([128, 4], f32, tag="racc")
        nc.vector.tensor_reduce(
            raccn[:].unsqueeze(2), zzn[:], mybir.AxisListType.X, OP.add
        )
        nc.vector.tensor_tensor(accN[:], accN[:], raccn[:], OP.add)

        # ---------------- finalize: ship [128,4] partials; host sums -------
        accT = const.tile([128, 4], f32)
        nc.vector.tensor_tensor(accT[:], accE[:], accN[:], OP.add)
        nc.sync.dma_start(out_d.ap(), accT[:])

    nc.compile()
    return nc


def _shard(inputs):
    f16 = np.float16

    node_feats = np.asarray(inputs["node_feats"], np.float32)
    edge_feats = np.asarray(inputs["edge_feats"], np.float32)
    Z = np.asarray(inputs["atomic_numbers"], np.int64)
    idx_s = np.asarray(inputs["idx_s"], np.int32)
    idx_t = np.asarray(inputs["idx_t"], np.int32)
    batch = np.asarray(inputs["batch"], np.int32)

    bounds = np.searchsorted(batch, np.arange(NUM_GRAPHS + 1)).astype(np.int64)
    g_t = batch[idx_t]
    core_of_edge = (g_t >> 2).astype(np.int32)

    e_counts = np.bincount(core_of_edge, minlength=NCORES)
    ET = int(-(-e_counts.max() // UNIT) * UNIT)
    n_counts = bounds[4 * np.arange(NCORES) + 4] - bounds[4 * np.arange(NCORES)]
    NT = int(-(-n_counts.max() // UNIT) * UNIT)
    NGRP = -(-ET // GROUP)
    NSW = -(-(NT // SUPER) // 8)

    ascale = np.asarray(inputs["atom_scales"], np.float32)[:, 0]
    ashift = np.asarray(inputs["atom_shifts"], np.float32)[:, 0]
    pair = np.asarray(inputs["pair_scales"], np.float32)[:, 0]

    W1e = np.asarray(inputs["W1e"], np.float32).astype(f16)
    b1e = np.asarray(inputs["b1e"], np.float32).reshape(128, 1)
    W2e = np.tile(np.asarray(inputs["W2e"], np.float32).reshape(128, 1),
                  (1, 32)).astype(f16)
    W1n = np.asarray(inputs["W1n"], np.float32).astype(f16)
    b1n = np.ascontiguousarray(np.asarray(inputs["b1n"], np.float32).reshape(2, 128).T)
    W2n_2 = np.asarray(inputs["W2n"], np.float32).reshape(2, 128).T
    W2n = np.concatenate(
        [np.tile(W2n_2[:, 0:1], (1, 32)), np.tile(W2n_2[:, 1:2], (1, 32))], axis=1
    ).astype(f16)
    b2 = np.tile(np.array(
        [[np.asarray(inputs["b2e"], np.float32)[0],
          np.asarray(inputs["b2n"], np.float32)[0]]], np.float32), (128, 1))

    # per-edge coefficient (host table lookup; see module docstring)
    c_all = (pair[Z[idx_s] * NZ + Z[idx_t]] * ascale[Z[idx_t]]).astype(np.float32)

    order = np.argsort(core_of_edge, kind="stable")
    starts = np.searchsorted(core_of_edge, np.arange(NCORES + 1), sorter=order)

    in_maps = []
    for k in range(NCORES):
        n0 = int(bounds[4 * k])
        n1 = int(bounds[4 * k + 4])
        nn = n1 - n0
        sel = order[starts[k]:starts[k + 1]]
        E = sel.size

        eTk = np.zeros((D_EDGE, ET), f16)
        eTk[:, :E] = edge_feats[sel].T
        cpad = np.zeros(NGRP * GROUP, np.float32)
        cpad[:E] = c_all[sel]
        itw = np.full(NGRP * GROUP, PAD_I, np.int32)
        itw[:E] = idx_t[sel]

        nTk = np.zeros((D_NODE, NT), f16)
        nTk[:, :nn] = node_feats[n0:n1].T
        NTW = NSW * SWEEP
        apad = np.zeros(NTW, np.float32)
        apad[:nn] = ascale[Z[n0:n1]]
        hpad = np.zeros(NTW, np.float32)
        hpad[:nn] = ashift[Z[n0:n1]]
        idn = np.full(NTW, PAD_I, np.int32)
        idn[:nn] = np.arange(nn, dtype=np.int32)

        Brow = bounds[[4 * k + 1, 4 * k + 2, 4 * k + 3, 4 * k + 4]].astype(np.int32)
        in_maps.append({
            "eT": eTk,
            "CL": _group_layout(cpad, NGRP),
            "itwL": _group_layout(itw, NGRP),
            "Brow": np.tile(Brow.reshape(1, 4), (128, 1)),
            "nTa": np.ascontiguousarray(nTk[:128]),
            "nTb": np.ascontiguousarray(nTk[128:]),
            "AL": _sweep_layout(apad, NSW),
            "HL": _sweep_layout(hpad, NSW),
            "idnL": _sweep_layout(idn, NSW),
            "BrowL": np.tile((Brow - n0).reshape(1, 4), (128, 1)),
            "W1e": W1e, "b1e": b1e, "W2e": W2e,
            "W1n": W1n, "b1n": b1n, "W2n": W2n, "b2": b2,
        })
    return ET, NT, in_maps


LAST_RES = None


def kernel(**inputs) -> np.ndarray:
    global LAST_RES
    from concourse.bass_utils import run_bass_kernel_spmd

    ET, NT, in_maps = _shard(inputs)
    key = (ET, NT)
    if key not in _CACHE:
        _CACHE[key] = _build(ET, NT)
    nc = _CACHE[key]

    res = run_bass_kernel_spmd(nc, in_maps, core_ids=list(range(NCORES)))
    LAST_RES = res
    Y = np.zeros(NUM_GRAPHS, np.float32)
    for k in range(NCORES):
        yp = np.asarray(res.results[k]["out"]).reshape(128, 4).sum(axis=0)
        Y[4 * k] = yp[0]
        Y[4 * k + 1] = yp[1] - yp[0]
        Y[4 * k + 2] = yp[2] - yp[1]
        Y[4 * k + 3] = yp[3] - yp[2]
    return Y
